# Initial kernel scaffold
#
"""Trainium2 Bass kernel for nn_ATC_Network (2-layer GCN + BN + LeakyReLU).

Computation (see reference):
    row, col, w  (+ self loops w=1)
    deg[c] = sum_{e: col=c} w_e ;  dis = rsqrt(deg)
    norm_e = dis[row]*w*dis[col]
    z1[c]  = sum_e norm_e * x[row]          (conv1 aggregate, incl self loop)
    y1     = z1 @ W1 + b1 ; x2 = LeakyReLU(BN(y1))
    z2[c]  = sum_e norm_e * x2[row]
    y2     = z2 @ W2 + b2 ; out = BN(y2)

Sharding: nodes split into 8 contiguous ranges (one per core).  Each core owns
all edges targeting its nodes.  Host pre-sorts edges per (core, src-half,
dest-tile, dest) into padded 128-slot blocks with *uniform* shapes across
cores (SPMD: one instruction stream).  Device: dma_gather fetches pre-scaled
source rows x' = dis*x, a weighted one-hot (built on DVE from host metadata)
reduces each 128-slot block into PSUM via the tensor engine.  BN stats are
computed from second-moment matrices (z'z) folded through W, AllReduce'd.
Layer-2 features are exchanged with an AllGather.
"""

import sys

sys.path.insert(0, "/opt/trn_rl_repo")

import numpy as np
import ml_dtypes

import concourse.bass as bass
import concourse.tile as tile
from concourse import bacc, bass_utils, mybir
from concourse.masks import make_identity

FP32 = mybir.dt.float32
F32R = mybir.dt.float32r
F16 = mybir.dt.float16
BF16 = mybir.dt.bfloat16
I16 = mybir.dt.int16

# ---------------------------------------------------------------- config ----
CFG = dict(
    N=50000, E=800000, F=128, H=128, O=64, NCORE=8,
    HALF=25000,          # src split point for int16 gather indices
    GT=3,                # dest tiles per gather group
    YC=512,              # node columns per y-matmul chunk
    EPS=1e-5, NEG=0.01,
)


# ---------------------------------------------------------- preprocessing ---
def preprocess(adj, w, cfg):
    """Build per-core slot streams + uniform block structure (numpy only)."""
    N, E, NCORE, HALF = cfg["N"], cfg["E"], cfg["NCORE"], cfg["HALF"]
    NP = N // NCORE
    T = (NP + 127) // 128
    LT = NP - 128 * (T - 1)

    row = np.asarray(adj[0], np.int64)
    col = np.asarray(adj[1], np.int64)
    w = np.asarray(w, np.float32)

    core = col // NP
    lc = col % NP
    tl = lc // 128
    dl = lc % 128
    half = (row >= HALF).astype(np.int64)

    # stable sort by (core, half, tile, dest)
    key = ((core * 2 + half) * T + tl) * 128 + dl
    order = np.argsort(key, kind="stable")
    skey = key[order]

    # rank of each edge within its (core, half, tile, dest) group
    grp_start = np.r_[0, np.flatnonzero(np.diff(skey)) + 1]
    grp_id = np.cumsum(np.r_[0, np.diff(skey) != 0])
    rank_in_dest = np.arange(E) - grp_start[grp_id]

    # counts per (core, half, tile)
    cht = (core * 2 + half) * T + tl
    cnt_cht = np.bincount(cht, minlength=NCORE * 2 * T).reshape(NCORE, 2, T)
    cap = np.maximum(cnt_cht.max(0), 1)                     # [2, T]
    cap = ((cap + 127) // 128) * 128
    tsb = np.zeros((2, T + 1), np.int64)                    # slot base per tile
    tsb[:, 1:] = np.cumsum(cap, 1)
    L = tsb[:, -1].copy()                                   # stream length/half
    nblk_half = (L // 128).astype(np.int64)
    NBA, NBB = int(nblk_half[0]), int(nblk_half[1])
    NB = NBA + NBB

    # per-edge rank within (core, half, tile) group -> stream slot
    ct_key = cht  # groups already contiguous in sorted order
    sk2 = ct_key[order]
    g2_start = np.r_[0, np.flatnonzero(np.diff(sk2)) + 1]
    g2_id = np.cumsum(np.r_[0, np.diff(sk2) != 0])
    rank_in_ct = np.arange(E) - g2_start[g2_id]
    slot = tsb[half[order], tl[order]] + rank_in_ct         # slot within half-stream

    # fill per-core streams (flat per (core, half))
    idx_stream = [[np.zeros(int(L[h]), np.int32) for h in range(2)] for _ in range(NCORE)]
    w_stream = [[np.zeros(int(L[h]), np.float32) for h in range(2)] for _ in range(NCORE)]
    d_stream = [[np.zeros(int(L[h]), np.int32) for h in range(2)] for _ in range(NCORE)]
    oc, oh = core[order], half[order]
    orow, ow, odl = row[order], w[order], dl[order]
    for c in range(NCORE):
        for h in range(2):
            m = (oc == c) & (oh == h)
            s = slot[m]
            idx_stream[c][h][s] = orow[m] - h * HALF
            w_stream[c][h][s] = ow[m]
            d_stream[c][h][s] = odl[m]

    # uniform block structure: d0 / width per 128-slot block (union over cores)
    d0 = np.zeros((2, max(NBA, NBB)), np.int64)
    dend = np.ones((2, max(NBA, NBB)), np.int64)
    for h, nb in ((0, NBA), (1, NBB)):
        dmin = np.full((nb,), 1 << 30, np.int64)
        dmax = np.full((nb,), -1, np.int64)
        for c in range(NCORE):
            dd = d_stream[c][h].reshape(nb, 128)
            ww = w_stream[c][h].reshape(nb, 128)
            real = ww > 0
            any_real = real.any(1)
            dmasked = np.where(real, dd, 1 << 30)
            dmin2 = dmasked.min(1)
            dmasked = np.where(real, dd, -1)
            dmax2 = dmasked.max(1)
            dmin = np.where(any_real, np.minimum(dmin, dmin2), dmin)
            dmax = np.where(any_real, np.maximum(dmax, dmax2), dmax)
        none = dmax < 0
        dmin[none] = 0
        dmax[none] = 0
        # fp32r matmul ISA: even output column offset and even width
        dmin = dmin & ~1
        wid = dmax + 1 - dmin
        wid = np.minimum((wid + 1) & ~1, 128 - dmin)
        d0[h, :nb] = dmin
        dend[h, :nb] = dmin + wid
    Cu = int((dend - d0).max())
    Cu = max(Cu, 2)

    # block -> tile map (per half)
    blk_tile = np.zeros((2, max(NBA, NBB)), np.int64)
    for h in range(2):
        for t in range(T):
            blk_tile[h, tsb[h, t] // 128: tsb[h, t + 1] // 128] = t

    # comparand [NB, Cu] (A blocks then B blocks) -- same for all cores
    cmp = np.zeros((NB, Cu), np.float32)
    cmp[:NBA] = d0[0, :NBA, None] + np.arange(Cu)[None]
    cmp[NBA:] = d0[1, :NBB, None] + np.arange(Cu)[None]
    cmp_bf = cmp.astype(ml_dtypes.bfloat16)

    # per-core destid/w in [128, NB] block layout
    destid = np.zeros((NCORE, 128, NB), np.float32)
    wval = np.zeros((NCORE, 128, NB), np.float32)
    for c in range(NCORE):
        a = d_stream[c][0].reshape(NBA, 128).T
        b = d_stream[c][1].reshape(NBB, 128).T
        destid[c, :, :NBA] = a
        destid[c, :, NBA:] = b
        a = w_stream[c][0].reshape(NBA, 128).T
        b = w_stream[c][1].reshape(NBB, 128).T
        wval[c, :, :NBA] = a
        wval[c, :, NBA:] = b
    destid_bf = destid.astype(ml_dtypes.bfloat16)

    # idx arrays, wrapped [16, L/16] then replicated to 128 partitions
    def wrap_idx(a):
        v = a.astype(np.int16).reshape(-1, 16).T  # [16, L/16]
        return np.tile(v, (8, 1))                 # [128, L/16]

    idxA = np.stack([wrap_idx(idx_stream[c][0]) for c in range(NCORE)])
    idxB = np.stack([wrap_idx(idx_stream[c][1]) for c in range(NCORE)])

    # padded per-dest weight table for degree: [128, sum_t (D_t+1)]
    degc = np.zeros((NCORE, T, 128), np.int64)
    np.add.at(degc, (core, tl, dl), 1)
    Dt = degc.max(0).max(1)           # [T]
    Wt = Dt + 1
    woff = np.zeros(T + 1, np.int64)
    woff[1:] = np.cumsum(Wt)
    W2TOT = int(woff[-1])
    w2 = np.zeros((NCORE, 128, W2TOT), np.float32)
    # per-edge rank within (core, tile, dest) over BOTH halves:
    key3 = (core * T + tl) * 128 + dl
    order3 = np.argsort(key3, kind="stable")
    sk3 = key3[order3]
    g3_start = np.r_[0, np.flatnonzero(np.diff(sk3)) + 1]
    g3_id = np.cumsum(np.r_[0, np.diff(sk3) != 0])
    rank3 = np.arange(E) - g3_start[g3_id]
    w2[core[order3], dl[order3], woff[tl[order3]] + rank3] = w[order3]
    w2[:, :, woff[1:] - 1] = 1.0  # self-loop column per tile (all partitions)

    # gather groups (GT tiles each)
    GT = cfg["GT"]
    groups = []
    for g0 in range(0, T, GT):
        g1 = min(g0 + GT, T)
        ent = dict(t0=g0, t1=g1)
        for h, tag in ((0, "A"), (1, "B")):
            s0, s1 = int(tsb[h, g0]), int(tsb[h, g1])
            ent[f"s0{tag}"], ent[f"s1{tag}"] = s0, s1
            ent[f"b0{tag}"], ent[f"b1{tag}"] = s0 // 128, s1 // 128
        groups.append(ent)

    pad_ratio = (L[0] + L[1]) / max(1.0, E / NCORE)
    return dict(
        cfg=cfg, NP=NP, T=T, LT=LT, NBA=NBA, NBB=NBB, NB=NB, Cu=Cu,
        LA=int(L[0]), LB=int(L[1]), tsb=tsb, d0=d0, dend=dend,
        blk_tile=blk_tile, groups=groups, cmp=cmp_bf, destid=destid_bf,
        wval=wval, idxA=idxA, idxB=idxB, w2=w2, woff=woff, Wt=Wt,
        W2TOT=W2TOT, pad_ratio=float(pad_ratio),
    )


# ------------------------------------------------------------ bass program --
STAGES = ["dis", "xprime", "conv1nr", "conv1", "bn1", "ag2", "conv2nr", "full"]
DBG = dict(no_gather=False, no_wsel=False, no_blocks=False, no_self=False,
           no_moments=False, no_close=False)


def build(st, stage="full", reps=1):
    slev = STAGES.index(stage)
    cfg = st["cfg"]
    N, F, H, O, NCORE = cfg["N"], cfg["F"], cfg["H"], cfg["O"], cfg["NCORE"]
    HALF, EPS, NEG, YC = cfg["HALF"], cfg["EPS"], cfg["NEG"], cfg["YC"]
    NP, T, LT, NB, NBA, NBB, Cu = (st["NP"], st["T"], st["LT"], st["NB"],
                                   st["NBA"], st["NBB"], st["Cu"])
    W2TOT, woff, Wt = st["W2TOT"], st["woff"], st["Wt"]
    d0s, dends, blk_tile, tsb = st["d0"], st["dend"], st["blk_tile"], st["tsb"]
    HB = N - HALF
    rg = [list(range(NCORE))]

    nc = bacc.Bacc("TRN2", target_bir_lowering=False, debug=False,
                   num_devices=NCORE, num_swdge_queues=4)
    GCH = 1024  # max dma_gather indices per instruction (desc ring capacity)
    NQ = 4
    qctr = [0]  # rotate swdge queues so desc-gen overlaps draining

    # --- I/O ---
    x_full = nc.dram_tensor("x_full", [N, F], FP32, kind="ExternalInput")
    x_own = nc.dram_tensor("x_own", [NP, F], FP32, kind="ExternalInput")
    W1 = nc.dram_tensor("w1", [F, H], FP32, kind="ExternalInput")
    g1 = nc.dram_tensor("g1", [H], FP32, kind="ExternalInput")
    be1 = nc.dram_tensor("beta1", [H], FP32, kind="ExternalInput")
    W2 = nc.dram_tensor("w2m", [H, O], FP32, kind="ExternalInput")
    g2 = nc.dram_tensor("g2", [O], FP32, kind="ExternalInput")
    be2 = nc.dram_tensor("beta2", [O], FP32, kind="ExternalInput")
    idxA_d = nc.dram_tensor("idxA", [128, st["LA"] // 16], I16, kind="ExternalInput")
    idxB_d = nc.dram_tensor("idxB", [128, st["LB"] // 16], I16, kind="ExternalInput")
    did_d = nc.dram_tensor("destid", [128, NB], BF16, kind="ExternalInput")
    wv_d = nc.dram_tensor("wval", [128, NB], FP32, kind="ExternalInput")
    cmp_d = nc.dram_tensor("cmp", [NB, Cu], BF16, kind="ExternalInput")
    w2_d = nc.dram_tensor("w2t", [128, W2TOT], FP32, kind="ExternalInput")
    out_d = nc.dram_tensor("out", [NP, O], FP32, kind="ExternalOutput")

    def bcast_inner(ap, k):
        return bass.AP(tensor=ap.tensor, offset=ap.offset, ap=ap.ap + [[0, k]])

    def bcast_part(src_ap, off, n, parts=128):
        return bass.AP(tensor=src_ap.tensor, offset=src_ap.offset + off,
                       ap=[[0, parts], [1, n]])

    with tile.TileContext(nc) as tc:
        sing = tc.alloc_tile_pool(name="sing", bufs=1)
        small = tc.alloc_tile_pool(name="small", bufs=3)
        gbufA_p = tc.alloc_tile_pool(name="gbufA", bufs=2)
        gbufB_p = tc.alloc_tile_pool(name="gbufB", bufs=2)
        wselp = tc.alloc_tile_pool(name="wselp", bufs=2)
        big = tc.alloc_tile_pool(name="big", bufs=1)
        xsc = tc.alloc_tile_pool(name="xsc", bufs=3)
        ptile = tc.alloc_tile_pool(name="ptile", bufs=2, space="PSUM")
        pmisc = tc.alloc_tile_pool(name="pmisc", bufs=2, space="PSUM")
        pfold = tc.alloc_tile_pool(name="pfold", bufs=1, space="PSUM")
        pmom = tc.alloc_tile_pool(name="pmom", bufs=1, space="PSUM")
        pyc = tc.alloc_tile_pool(name="pyc", bufs=2, space="PSUM")
        dram = tc.alloc_tile_pool(name="dram", bufs=1, space="DRAM")

        # --- persistent DRAM scratch ---
        x_prime = dram.tile([N, F], F16)
        din = dram.tile([((NP + 127) // 128) * 128], FP32)
        dago = dram.tile([((NP + 127) // 128) * 128 * NCORE], FP32)
        dis_full = dram.tile([N], FP32)
        mo_in = dram.tile([128, H + 1], FP32)
        mo_out = dram.tile([128, H + 1], FP32)
        mo2_in = dram.tile([128, H + 1], FP32)
        mo2_out = dram.tile([128, H + 1], FP32)
        x2own_d = dram.tile([NP, F], F16)
        x2full = dram.tile([N, F], F16)

        # --- constants ---
        ident = sing.tile([128, 128], FP32)
        make_identity(nc, ident[:])
        ones_col = sing.tile([128, 1], FP32)
        nc.vector.memset(ones_col[:], 1.0)
        ident_h = sing.tile([128, 128], F16)
        nc.vector.tensor_copy(out=ident_h[:], in_=ident[:])
        zero128 = sing.tile([128, 128], FP32)
        nc.vector.memset(zero128[:], 0.0)
        eps_sb = sing.tile([128, 1], FP32)
        nc.vector.memset(eps_sb[:], EPS)

        W1_sb = sing.tile([F, H], FP32)
        nc.sync.dma_start(out=W1_sb[:], in_=W1[:, :])
        W2_sb = sing.tile([H, O], FP32)
        nc.sync.dma_start(out=W2_sb[:], in_=W2[:, :])
        g1_sb = sing.tile([H, 1], FP32)
        nc.sync.dma_start(out=g1_sb[:], in_=g1[:, None])
        be1_sb = sing.tile([H, 1], FP32)
        nc.sync.dma_start(out=be1_sb[:], in_=be1[:, None])
        g2_sb = sing.tile([O, 1], FP32)
        nc.sync.dma_start(out=g2_sb[:], in_=g2[:, None])
        be2_sb = sing.tile([O, 1], FP32)
        nc.sync.dma_start(out=be2_sb[:], in_=be2[:, None])

        def emit_once():
            # =============== stage 1: degree -> dis ===============
            w2_sb = sing.tile([128, W2TOT], FP32)
            nc.sync.dma_start(out=w2_sb[:], in_=w2_d[:, :])
            deg_sb = sing.tile([128, T], FP32)
            for t in range(T):
                nc.vector.tensor_reduce(
                    out=deg_sb[:, t:t + 1],
                    in_=w2_sb[:, int(woff[t]):int(woff[t + 1])],
                    axis=mybir.AxisListType.X, op=mybir.AluOpType.add)
            sq_sb = sing.tile([128, T], FP32)
            nc.scalar.activation(out=sq_sb[:], in_=deg_sb[:],
                                 func=mybir.ActivationFunctionType.Sqrt)
            dis_sb = sing.tile([128, T], FP32)
            nc.vector.reciprocal(out=dis_sb[:], in_=sq_sb[:])
            # write node-ordered dis_own -> din (via PE transpose: few descs)
            pdt = pmisc.tile([128, 128], FP32, tag="ptr")
            nc.tensor.transpose(pdt[:T, :], dis_sb[:, :T], ident[:])
            disrow = small.tile([128, 128], FP32, tag="disrow")
            nc.vector.tensor_copy(out=disrow[:T], in_=pdt[:T, :])
            if T > 1:
                nc.sync.dma_start(
                    out=bass.AP(tensor=din[:].tensor, offset=din[:].offset,
                                ap=[[128, T - 1], [1, 128]]),
                    in_=disrow[:T - 1, :])
            nc.sync.dma_start(
                out=bass.AP(tensor=din[:].tensor,
                            offset=din[:].offset + 128 * (T - 1),
                            ap=[[0, 1], [1, LT]]),
                in_=disrow[T - 1:T, :LT])
            # zero the tail pad of din
            padn = din.shape[0] - NP
            if padn > 0:
                nc.sync.dma_start(
                    out=bass.AP(tensor=din[:].tensor, offset=din[:].offset + NP,
                                ap=[[1, padn]]),
                    in_=zero128[0:1, :padn])

            nc.gpsimd.collective_compute(
                "AllGather", mybir.AluOpType.bypass, replica_groups=rg,
                ins=[din.opt()], outs=[dago.opt()])
            # repack (strip per-rank pad) into dis_full
            PADP = din.shape[0]
            for c in range(NCORE):
                nc.sync.dma_start(
                    out=bass.AP(tensor=dis_full[:].tensor,
                                offset=dis_full[:].offset + c * NP, ap=[[1, NP]]),
                    in_=bass.AP(tensor=dago[:].tensor,
                                offset=dago[:].offset + c * PADP, ap=[[1, NP]]))

            # disB: [128, NP] broadcast of own dis (for feat-major per-node mults)
            disB = big.tile([128, NP], FP32)
            nc.sync.dma_start(out=disB[:], in_=bcast_part(din[:], 0, NP))

            # dis_sbT [128, NR]: column m holds dis[m*128 + p] (for x'-scale)
            NR = (N + 127) // 128
            LROW = N - 128 * (NR - 1)
            dis_sbT = sing.tile([128, NR], FP32)
            for r in range(0, NR, 128):
                rows = min(128, NR - r)
                full_rows = rows if (r + rows < NR or LROW == 128) else rows - 1
                mt = small.tile([128, 128], FP32, tag="dmt")
                if rows < 128 or full_rows < rows:
                    nc.vector.memset(mt[:], 1.0)
                if full_rows > 0:
                    nc.sync.dma_start(
                        out=mt[:full_rows],
                        in_=bass.AP(tensor=dis_full[:].tensor,
                                    offset=dis_full[:].offset + r * 128,
                                    ap=[[128, full_rows], [1, 128]]))
                if full_rows < rows:
                    nc.sync.dma_start(
                        out=mt[full_rows:full_rows + 1, :LROW],
                        in_=bass.AP(tensor=dis_full[:].tensor,
                                    offset=dis_full[:].offset
                                    + (r + full_rows) * 128,
                                    ap=[[0, 1], [1, LROW]]))
                pdr = pmisc.tile([128, 128], FP32, tag="ptr")
                nc.tensor.transpose(pdr[:], mt[:], ident[:])
                nc.vector.tensor_copy(out=dis_sbT[:, r:r + rows],
                                      in_=pdr[:, :rows])

            # =============== stage 2: x' = x * dis ===============
            ng512 = (N // 512) if slev >= 1 else 0
            for gidx in range(ng512):
                base = gidx * 512
                xt = xsc.tile([128, 4, 128], FP32)
                src = bass.AP(tensor=x_full.ap().tensor, offset=base * F,
                              ap=[[F, 128], [128 * F, 4], [1, F]])
                nc.sync.dma_start(out=xt[:], in_=src)
                xt16 = xsc.tile([128, 4, 128], F16, tag="xt16")
                nc.vector.tensor_tensor(
                    out=xt16[:], in0=xt[:],
                    in1=bcast_inner(dis_sbT[:, gidx * 4:gidx * 4 + 4], 128),
                    op=mybir.AluOpType.mult)
                dst = bass.AP(tensor=x_prime[:].tensor,
                              offset=x_prime[:].offset + base * F,
                              ap=[[F, 128], [128 * F, 4], [1, F]])
                nc.sync.dma_start(out=dst, in_=xt16[:])
            rem = (N - (N // 512) * 512) if slev >= 1 else 0
            base = (N // 512) * 512
            while rem > 0:
                nn_ = min(128, rem)
                xt = xsc.tile([128, 128], FP32)
                nc.sync.dma_start(out=xt[:nn_], in_=x_full[base:base + nn_, :])
                col = base // 128
                xt16 = xsc.tile([128, 128], F16, tag="xt16t")
                nc.vector.tensor_scalar_mul(out=xt16[:nn_], in0=xt[:nn_],
                                            scalar1=dis_sbT[:nn_, col:col + 1])
                nc.sync.dma_start(out=x_prime[base:base + nn_, :], in_=xt16[:nn_])
                base += nn_
                rem -= nn_

            # =============== conv layer (shared emitter) ===============
            def conv(layer, src_dram, self_dram, self_scale, zT, do_ar=True):
                """Aggregate z^T[feat, own-node] * dis  into zT (sbuf [F, NP])."""
                Mp = pmom.tile([128, H + 1], FP32)  # moments accumulate (F x F | sum)
                for g in st["groups"]:
                    t0, t1 = g["t0"], g["t1"]
                    bufs = {}
                    for h, tag, pool, idx_d, Lh in ((0, "A", gbufA_p, idxA_d, st["LA"]),
                                                    (1, "B", gbufB_p, idxB_d, st["LB"])):
                        s0, s1 = g[f"s0{tag}"], g[f"s1{tag}"]
                        nb = (s1 - s0) // 128
                        if nb == 0:
                            continue
                        it = small.tile([128, (s1 - s0) // 16], I16, tag=f"idx{tag}")
                        nc.sync.dma_start(out=it[:], in_=idx_d[:, s0 // 16:s1 // 16])
                        gb = pool.tile([128, nb, 128], F16, tag=f"g{tag}")
                        if h == 0:
                            src_ap = src_dram[0:HALF, :]
                        else:
                            src_ap = src_dram[HALF:N, :]
                        if not DBG["no_gather"]:
                            for o in range(0, s1 - s0, GCH):
                                ni = min(GCH, s1 - s0 - o)
                                nc.gpsimd.dma_gather(
                                    out_ap=gb[:, o // 128:(o + ni) // 128, :],
                                    in_ap=src_ap,
                                    idxs_ap=it[:, o // 16:(o + ni) // 16],
                                    num_idxs=ni, num_idxs_reg=ni, elem_size=F,
                                    queue_num=qctr[0] % NQ)
                                qctr[0] += 1
                        else:
                            nc.vector.memset(gb[:], 0.25)
                        # block metadata for this span
                        cb0 = g[f"b0{tag}"] + (0 if h == 0 else NBA)
                        cb1 = g[f"b1{tag}"] + (0 if h == 0 else NBA)
                        dd = small.tile([128, nb], BF16, tag=f"dd{tag}")
                        nc.sync.dma_start(out=dd[:], in_=did_d[:, cb0:cb1])
                        wv = small.tile([128, nb], FP32, tag=f"wv{tag}")
                        nc.sync.dma_start(out=wv[:], in_=wv_d[:, cb0:cb1])
                        cm = small.tile([128, nb, Cu], BF16, tag=f"cm{tag}")
                        nc.sync.dma_start(
                            out=cm[:],
                            in_=bcast_part(cmp_d[:, :], cb0 * Cu, nb * Cu))
                        ws = wselp.tile([128, nb, Cu], F16, tag=f"ws{tag}")
                        if not DBG["no_wsel"]:
                            nc.vector.tensor_tensor(out=ws[:], in0=cm[:],
                                                    in1=bcast_inner(dd[:], Cu),
                                                    op=mybir.AluOpType.is_equal)
                            nc.vector.tensor_tensor(out=ws[:], in0=ws[:],
                                                    in1=bcast_inner(wv[:], Cu),
                                                    op=mybir.AluOpType.mult)
                        else:
                            nc.vector.memset(ws[:], 0.5)
                        bufs[h] = (gb, ws, g[f"b0{tag}"], nb)

                    for t in range(t0, t1):
                        tn = 128 if t < T - 1 else LT
                        blist = []
                        for h in (0, 1):
                            if h not in bufs:
                                continue
                            gb, ws, bbase, nb = bufs[h]
                            for b in range(int(tsb[h, t]) // 128,
                                           int(tsb[h, t + 1]) // 128):
                                blist.append((gb, ws, b - bbase,
                                              int(d0s[h, b]),
                                              min(int(dends[h, b] - d0s[h, b]),
                                                  Cu, 128 - int(d0s[h, b]))))
                        pz = ptile.tile([128, 128], FP32, tag="pz")
                        # self loop term (start=True resets the accumulation)
                        sv = small.tile([128, 128], F16, tag="selfv")
                        if tn < 128:
                            nc.vector.memset(sv[:], 0.0)
                        if self_scale:
                            svf = small.tile([128, 128], FP32, tag="selfvf")
                            nc.sync.dma_start(
                                out=svf[:tn],
                                in_=self_dram[t * 128:t * 128 + tn, :])
                            nc.scalar.activation(
                                out=sv[:tn], in_=svf[:tn],
                                func=mybir.ActivationFunctionType.Identity,
                                scale=dis_sb[:tn, t:t + 1])
                        else:
                            nc.sync.dma_start(
                                out=sv[:tn],
                                in_=self_dram[t * 128:t * 128 + tn, :])
                        if DBG["no_blocks"]:
                            blist = []
                        if not DBG["no_self"]:
                            nc.tensor.matmul(pz[:], lhsT=sv[:], rhs=ident_h[:],
                                             start=True, stop=(len(blist) == 0),
                                             skip_group_check=True)
                        for i, (gb, ws, j, dd0, cb) in enumerate(blist):
                            nc.tensor.matmul(
                                pz[:, dd0:dd0 + cb],
                                lhsT=gb[:, j, :], rhs=ws[:, j, 0:cb],
                                start=(DBG["no_self"] and i == 0),
                                stop=(i == len(blist) - 1),
                                skip_group_check=True)
                        # close tile: zT[:, range] = psum * disB
                        if not DBG["no_close"]:
                            nc.vector.tensor_tensor(
                                out=zT[:, t * 128:t * 128 + tn], in0=pz[:, :tn],
                                in1=disB[:, t * 128:t * 128 + tn],
                                op=mybir.AluOpType.mult)
                        else:
                            nc.vector.memset(zT[:, t * 128:t * 128 + tn], 0.1)
                        if DBG["no_moments"]:
                            continue
                        # moments: transpose then M += z z^T, S += z^T 1
                        ptr = pmisc.tile([128, 128], FP32, tag="ptr")
                        nc.tensor.transpose(ptr[:tn, :],
                                            zT[:, t * 128:t * 128 + tn], ident[:])
                        zd = small.tile([128, H + 1], FP32, tag="zd")
                        if tn < 128:
                            nc.vector.memset(zd[:], 0.0)
                        nc.vector.memset(zd[:, H:H + 1], 1.0)
                        nc.vector.tensor_copy(out=zd[:tn, 0:H], in_=ptr[:tn, :])
                        nc.tensor.matmul(Mp[:, 0:H + 1], lhsT=zd[:, 0:H],
                                         rhs=zd[:, 0:H + 1],
                                         start=(t == 0), stop=(t == T - 1),
                                         skip_group_check=True)
                if DBG["no_moments"]:
                    return None
                mo_sb = small.tile([128, H + 1], FP32, tag="mo")
                nc.vector.tensor_copy(out=mo_sb[:], in_=Mp[:])
                min_d = mo_in if layer == 1 else mo2_in
                mout_d = mo_out if layer == 1 else mo2_out
                nc.sync.dma_start(out=min_d[:, :], in_=mo_sb[:])
                if not do_ar:
                    return None
                nc.gpsimd.collective_compute(
                    "AllReduce", mybir.AluOpType.add, replica_groups=rg,
                    ins=[min_d.opt()], outs=[mout_d.opt()])
                mg = small.tile([128, H + 1], FP32, tag="mg")
                nc.sync.dma_start(out=mg[:], in_=mout_d[:, :])
                return mg

            def bn_fold(mg, Wsb, HH, g_sb, be_sb):
                """-> (s, tb) per-feature scale/shift [HH, 1] from moments."""
                pf = pfold.tile([128, 128], FP32, tag="pf")
                # mu_lin = W^T S / N
                nc.tensor.matmul(pf[:HH, 0:1], lhsT=Wsb[:], rhs=mg[:, H:H + 1],
                                 start=True, stop=True, skip_group_check=True)
                mul_sb = small.tile([128, 1], FP32, tag="mul")
                nc.vector.tensor_scalar_mul(out=mul_sb[:HH], in0=pf[:HH, 0:1],
                                            scalar1=1.0 / N)
                # G = M W ; diag = sum_f W*G
                pg = pfold.tile([128, 128], FP32, tag="pf")
                nc.tensor.matmul(pg[:, 0:HH], lhsT=mg[:, 0:H], rhs=Wsb[:],
                                 start=True, stop=True, skip_group_check=True)
                wg = small.tile([128, HH], FP32, tag="wg")
                nc.vector.tensor_tensor(out=wg[:], in0=pg[:, 0:HH], in1=Wsb[:],
                                        op=mybir.AluOpType.mult)
                pd = pfold.tile([128, 128], FP32, tag="pf")
                nc.tensor.matmul(pd[:HH, 0:1], lhsT=wg[:], rhs=ones_col[:],
                                 start=True, stop=True, skip_group_check=True)
                var_sb = small.tile([128, 1], FP32, tag="var")
                nc.vector.tensor_scalar_mul(out=var_sb[:HH], in0=pd[:HH, 0:1],
                                            scalar1=1.0 / N)
                mu2 = small.tile([128, 1], FP32, tag="mu2")
                nc.vector.tensor_mul(mu2[:HH], mul_sb[:HH], mul_sb[:HH])
                nc.vector.tensor_sub(var_sb[:HH], var_sb[:HH], mu2[:HH])
                sqv = small.tile([128, 1], FP32, tag="sqv")
                nc.scalar.activation(out=sqv[:HH], in_=var_sb[:HH],
                                     func=mybir.ActivationFunctionType.Sqrt,
                                     bias=eps_sb[:HH])
                s_sb = small.tile([128, 1], FP32, tag="s")
                nc.vector.reciprocal(out=s_sb[:HH], in_=sqv[:HH])
                nc.vector.tensor_mul(s_sb[:HH], s_sb[:HH], g_sb[:HH])
                tb_sb = small.tile([128, 1], FP32, tag="tb")
                nc.vector.tensor_mul(tb_sb[:HH], mul_sb[:HH], s_sb[:HH])
                nc.vector.tensor_sub(tb_sb[:HH], be_sb[:HH], tb_sb[:HH])
                return s_sb, tb_sb

            # ---- layer 1 ----
            if slev >= 2:
                zT1 = big.tile([128, NP], FP32, tag="zbig")
                mg1 = conv(1, x_prime, x_own, True, zT1, do_ar=(slev >= 3))
            if slev >= 4:
                s1, tb1 = bn_fold(mg1, W1_sb, H, g1_sb, be1_sb)
            for c0 in range(0, NP, YC) if slev >= 4 else []:
                c1 = min(c0 + YC, NP)
                py = pyc.tile([128, YC], FP32, tag="py")
                nc.tensor.matmul(py[:, 0:c1 - c0], lhsT=W1_sb[:], rhs=zT1[:, c0:c1],
                                 start=True, stop=True, skip_group_check=True)
                u = small.tile([128, YC], FP32, tag="u")
                nc.scalar.activation(out=u[:, 0:c1 - c0], in_=py[:, 0:c1 - c0],
                                     func=mybir.ActivationFunctionType.Identity,
                                     scale=s1[:H], bias=tb1[:H])
                v = small.tile([128, YC], FP32, tag="v")
                nc.vector.tensor_scalar_mul(out=v[:, 0:c1 - c0], in0=u[:, 0:c1 - c0],
                                            scalar1=NEG)
                nc.vector.tensor_tensor(out=u[:, 0:c1 - c0], in0=u[:, 0:c1 - c0],
                                        in1=v[:, 0:c1 - c0], op=mybir.AluOpType.max)
                nc.vector.tensor_tensor(out=u[:, 0:c1 - c0], in0=u[:, 0:c1 - c0],
                                        in1=disB[:, c0:c1], op=mybir.AluOpType.mult)
                # x2' rows -> dram (AG input), per 128-node tile of this chunk
                for tb_ in range(c0 // 128, (c1 + 127) // 128):
                    n0 = tb_ * 128
                    tn = min(128, NP - n0)
                    ptr = pmisc.tile([128, 128], FP32, tag="ptr")
                    nc.tensor.transpose(ptr[:tn, :], u[:, n0 - c0:n0 - c0 + tn],
                                        ident[:])
                    xo = small.tile([128, 128], F16, tag="xo")
                    nc.vector.tensor_copy(out=xo[:tn], in_=ptr[:tn, :])
                    nc.sync.dma_start(out=x2own_d[n0:n0 + tn, :], in_=xo[:tn])
            if slev >= 5:
                nc.gpsimd.collective_compute(
                    "AllGather", mybir.AluOpType.bypass, replica_groups=rg,
                    ins=[x2own_d.opt()], outs=[x2full.opt()])

            # ---- layer 2 ----
            if slev >= 6:
                zT2 = big.tile([128, NP], FP32, tag="zbig")
                mg2 = conv(2, x2full, x2own_d, False, zT2, do_ar=(slev >= 7))
            if slev >= 7:
                s2, tb2 = bn_fold(mg2, W2_sb, O, g2_sb, be2_sb)
            for c0 in range(0, NP, YC) if slev >= 7 else []:
                c1 = min(c0 + YC, NP)
                py = pyc.tile([128, YC], FP32, tag="py")
                nc.tensor.matmul(py[:O, 0:c1 - c0], lhsT=W2_sb[:], rhs=zT2[:, c0:c1],
                                 start=True, stop=True, skip_group_check=True)
                u = small.tile([128, YC], FP32, tag="u")
                nc.scalar.activation(out=u[:O, 0:c1 - c0], in_=py[:O, 0:c1 - c0],
                                     func=mybir.ActivationFunctionType.Identity,
                                     scale=s2[:O], bias=tb2[:O])
                # transpose per 128-node tile and write out
                for tb_ in range(c0 // 128, (c1 + 127) // 128):
                    n0 = tb_ * 128
                    tn = min(128, NP - n0)
                    po = pmisc.tile([128, 128], FP32, tag="ptr")
                    nc.tensor.transpose(po[:tn, :O], u[:O, n0 - c0:n0 - c0 + tn],
                                        ident[:O, :O])
                    oo = small.tile([128, O], FP32, tag="oo")
                    nc.vector.tensor_copy(out=oo[:tn], in_=po[:tn, :O])
                    nc.sync.dma_start(out=out_d[n0:n0 + tn, :], in_=oo[:tn])


        for _rep in range(reps):
            emit_once()

        for p in (dram, pyc, pmom, pfold, pmisc, ptile, xsc, big, wselp,
                  gbufB_p, gbufA_p, small, sing):
            p.release()

    nc.compile()
    return nc


# ------------------------------------------------------------------ runner --
def make_in_maps(st, inputs):
    cfg = st["cfg"]
    N, NCORE = cfg["N"], cfg["NCORE"]
    NP = st["NP"]
    x = np.asarray(inputs["drug_smiles_fea"], np.float32)
    maps = []
    for c in range(NCORE):
        maps.append(dict(
            x_full=x,
            x_own=np.ascontiguousarray(x[c * NP:(c + 1) * NP]),
            w1=np.asarray(inputs["W1"], np.float32),
            g1=np.asarray(inputs["g1"], np.float32),
            beta1=np.asarray(inputs["beta1"], np.float32),
            w2m=np.asarray(inputs["W2"], np.float32),
            g2=np.asarray(inputs["g2"], np.float32),
            beta2=np.asarray(inputs["beta2"], np.float32),
            idxA=st["idxA"][c], idxB=st["idxB"][c],
            destid=np.ascontiguousarray(st["destid"][c]),
            wval=np.ascontiguousarray(st["wval"][c]),
            cmp=st["cmp"],
            w2t=np.ascontiguousarray(st["w2"][c]),
        ))
    return maps


_LAST = {}


def kernel(**inputs):
    cfg = CFG
    adj = np.asarray(inputs["ATC_adj"])
    w = np.asarray(inputs["ATC_weight"], np.float32)
    st = preprocess(adj, w, cfg)
    nc = build(st)
    maps = make_in_maps(st, inputs)
    res = bass_utils.run_bass_kernel_spmd(
        nc, maps, core_ids=list(range(cfg["NCORE"])))
    out = np.concatenate([res.results[c]["out"] for c in range(cfg["NCORE"])], 0)
    _LAST.update(st=st, nc=nc, maps=maps)
    return out



# revision 1
# speedup vs baseline: 1.2607x; 1.2607x over previous
"""Trainium2 Bass kernel for nn_ATC_Network (2-layer GCN + BN + LeakyReLU).

Computation (see reference):
    row, col, w  (+ self loops w=1)
    deg[c] = sum_{e: col=c} w_e ;  dis = rsqrt(deg)
    norm_e = dis[row]*w*dis[col]
    z1[c]  = sum_e norm_e * x[row]          (conv1 aggregate, incl self loop)
    y1     = z1 @ W1 + b1 ; x2 = LeakyReLU(BN(y1))
    z2[c]  = sum_e norm_e * x2[row]
    y2     = z2 @ W2 + b2 ; out = BN(y2)

Sharding: nodes split into 8 contiguous ranges (one per core).  Each core owns
all edges targeting its nodes.  Host pre-sorts edges per (core, src-half,
dest-tile, dest) into padded 128-slot blocks with *uniform* shapes across
cores (SPMD: one instruction stream).  Device: dma_gather fetches pre-scaled
source rows x' = dis*x, a weighted one-hot (built on DVE from host metadata)
reduces each 128-slot block into PSUM via the tensor engine.  BN stats are
computed from second-moment matrices (z'z) folded through W, AllReduce'd.
Layer-2 features are exchanged with an AllGather.
"""

import sys

sys.path.insert(0, "/opt/trn_rl_repo")

import numpy as np
import ml_dtypes

import concourse.bass as bass
import concourse.tile as tile
from concourse import bacc, bass_utils, mybir
from concourse.masks import make_identity

FP32 = mybir.dt.float32
F32R = mybir.dt.float32r
F16 = mybir.dt.float16
BF16 = mybir.dt.bfloat16
I16 = mybir.dt.int16

# ---------------------------------------------------------------- config ----
CFG = dict(
    N=50000, E=800000, F=128, H=128, O=64, NCORE=8,
    HALF=25000,          # src split point for int16 gather indices
    GT=3,                # dest tiles per gather group
    YC=512,              # node columns per y-matmul chunk
    EPS=1e-5, NEG=0.01,
)


# ---------------------------------------------------------- preprocessing ---
def preprocess(adj, w, cfg):
    """Build per-core slot streams + uniform block structure (numpy only)."""
    N, E, NCORE, HALF = cfg["N"], cfg["E"], cfg["NCORE"], cfg["HALF"]
    NP = N // NCORE
    T = (NP + 127) // 128
    LT = NP - 128 * (T - 1)

    row = np.asarray(adj[0], np.int64)
    col = np.asarray(adj[1], np.int64)
    w = np.asarray(w, np.float32)

    core = col // NP
    lc = col % NP
    tl = lc // 128
    dl = lc % 128
    half = (row >= HALF).astype(np.int64)

    # stable sort by (core, half, tile, dest)
    key = ((core * 2 + half) * T + tl) * 128 + dl
    order = np.argsort(key, kind="stable")
    skey = key[order]

    # rank of each edge within its (core, half, tile, dest) group
    grp_start = np.r_[0, np.flatnonzero(np.diff(skey)) + 1]
    grp_id = np.cumsum(np.r_[0, np.diff(skey) != 0])
    rank_in_dest = np.arange(E) - grp_start[grp_id]

    # counts per (core, half, tile)
    cht = (core * 2 + half) * T + tl
    cnt_cht = np.bincount(cht, minlength=NCORE * 2 * T).reshape(NCORE, 2, T)
    cap = np.maximum(cnt_cht.max(0), 1)                     # [2, T]
    cap = ((cap + 127) // 128) * 128
    tsb = np.zeros((2, T + 1), np.int64)                    # slot base per tile
    tsb[:, 1:] = np.cumsum(cap, 1)
    L = tsb[:, -1].copy()                                   # stream length/half
    nblk_half = (L // 128).astype(np.int64)
    NBA, NBB = int(nblk_half[0]), int(nblk_half[1])
    NB = NBA + NBB

    # per-edge rank within (core, half, tile) group -> stream slot
    ct_key = cht  # groups already contiguous in sorted order
    sk2 = ct_key[order]
    g2_start = np.r_[0, np.flatnonzero(np.diff(sk2)) + 1]
    g2_id = np.cumsum(np.r_[0, np.diff(sk2) != 0])
    rank_in_ct = np.arange(E) - g2_start[g2_id]
    slot = tsb[half[order], tl[order]] + rank_in_ct         # slot within half-stream

    # fill per-core streams (flat per (core, half))
    idx_stream = [[np.zeros(int(L[h]), np.int32) for h in range(2)] for _ in range(NCORE)]
    w_stream = [[np.zeros(int(L[h]), np.float32) for h in range(2)] for _ in range(NCORE)]
    d_stream = [[np.zeros(int(L[h]), np.int32) for h in range(2)] for _ in range(NCORE)]
    oc, oh = core[order], half[order]
    orow, ow, odl = row[order], w[order], dl[order]
    for c in range(NCORE):
        for h in range(2):
            m = (oc == c) & (oh == h)
            s = slot[m]
            idx_stream[c][h][s] = orow[m] - h * HALF
            w_stream[c][h][s] = ow[m]
            d_stream[c][h][s] = odl[m]

    # uniform block structure: d0 / width per 128-slot block (union over cores)
    d0 = np.zeros((2, max(NBA, NBB)), np.int64)
    dend = np.ones((2, max(NBA, NBB)), np.int64)
    for h, nb in ((0, NBA), (1, NBB)):
        dmin = np.full((nb,), 1 << 30, np.int64)
        dmax = np.full((nb,), -1, np.int64)
        for c in range(NCORE):
            dd = d_stream[c][h].reshape(nb, 128)
            ww = w_stream[c][h].reshape(nb, 128)
            real = ww > 0
            any_real = real.any(1)
            dmasked = np.where(real, dd, 1 << 30)
            dmin2 = dmasked.min(1)
            dmasked = np.where(real, dd, -1)
            dmax2 = dmasked.max(1)
            dmin = np.where(any_real, np.minimum(dmin, dmin2), dmin)
            dmax = np.where(any_real, np.maximum(dmax, dmax2), dmax)
        none = dmax < 0
        dmin[none] = 0
        dmax[none] = 0
        # fp32r matmul ISA: even output column offset and even width
        dmin = dmin & ~1
        wid = dmax + 1 - dmin
        wid = np.minimum((wid + 1) & ~1, 128 - dmin)
        d0[h, :nb] = dmin
        dend[h, :nb] = dmin + wid
    Cu = int((dend - d0).max())
    Cu = max(Cu, 2)

    # block -> tile map (per half)
    blk_tile = np.zeros((2, max(NBA, NBB)), np.int64)
    for h in range(2):
        for t in range(T):
            blk_tile[h, tsb[h, t] // 128: tsb[h, t + 1] // 128] = t

    # comparand [NB, Cu] (A blocks then B blocks) -- same for all cores
    cmp = np.zeros((NB, Cu), np.float32)
    cmp[:NBA] = d0[0, :NBA, None] + np.arange(Cu)[None]
    cmp[NBA:] = d0[1, :NBB, None] + np.arange(Cu)[None]
    cmp_bf = cmp.astype(ml_dtypes.bfloat16)

    # per-core destid/w in [128, NB] block layout
    destid = np.zeros((NCORE, 128, NB), np.float32)
    wval = np.zeros((NCORE, 128, NB), np.float32)
    for c in range(NCORE):
        a = d_stream[c][0].reshape(NBA, 128).T
        b = d_stream[c][1].reshape(NBB, 128).T
        destid[c, :, :NBA] = a
        destid[c, :, NBA:] = b
        a = w_stream[c][0].reshape(NBA, 128).T
        b = w_stream[c][1].reshape(NBB, 128).T
        wval[c, :, :NBA] = a
        wval[c, :, NBA:] = b
    destid_bf = destid.astype(ml_dtypes.bfloat16)

    # idx arrays, wrapped [16, L/16] then replicated to 128 partitions
    def wrap_idx(a):
        v = a.astype(np.int16).reshape(-1, 16).T  # [16, L/16]
        return np.tile(v, (8, 1))                 # [128, L/16]

    idxA = np.stack([wrap_idx(idx_stream[c][0]) for c in range(NCORE)])
    idxB = np.stack([wrap_idx(idx_stream[c][1]) for c in range(NCORE)])

    # padded per-dest weight table for degree: [128, sum_t (D_t+1)]
    degc = np.zeros((NCORE, T, 128), np.int64)
    np.add.at(degc, (core, tl, dl), 1)
    Dt = degc.max(0).max(1)           # [T]
    Wt = Dt + 1
    woff = np.zeros(T + 1, np.int64)
    woff[1:] = np.cumsum(Wt)
    W2TOT = int(woff[-1])
    w2 = np.zeros((NCORE, 128, W2TOT), np.float32)
    # per-edge rank within (core, tile, dest) over BOTH halves:
    key3 = (core * T + tl) * 128 + dl
    order3 = np.argsort(key3, kind="stable")
    sk3 = key3[order3]
    g3_start = np.r_[0, np.flatnonzero(np.diff(sk3)) + 1]
    g3_id = np.cumsum(np.r_[0, np.diff(sk3) != 0])
    rank3 = np.arange(E) - g3_start[g3_id]
    w2[core[order3], dl[order3], woff[tl[order3]] + rank3] = w[order3]
    w2[:, :, woff[1:] - 1] = 1.0  # self-loop column per tile (all partitions)

    # gather groups (GT tiles each)
    GT = cfg["GT"]
    groups = []
    for g0 in range(0, T, GT):
        g1 = min(g0 + GT, T)
        ent = dict(t0=g0, t1=g1)
        for h, tag in ((0, "A"), (1, "B")):
            s0, s1 = int(tsb[h, g0]), int(tsb[h, g1])
            ent[f"s0{tag}"], ent[f"s1{tag}"] = s0, s1
            ent[f"b0{tag}"], ent[f"b1{tag}"] = s0 // 128, s1 // 128
        groups.append(ent)

    pad_ratio = (L[0] + L[1]) / max(1.0, E / NCORE)
    return dict(
        cfg=cfg, NP=NP, T=T, LT=LT, NBA=NBA, NBB=NBB, NB=NB, Cu=Cu,
        LA=int(L[0]), LB=int(L[1]), tsb=tsb, d0=d0, dend=dend,
        blk_tile=blk_tile, groups=groups, cmp=cmp_bf, destid=destid_bf,
        wval=wval, idxA=idxA, idxB=idxB, w2=w2, woff=woff, Wt=Wt,
        W2TOT=W2TOT, pad_ratio=float(pad_ratio),
    )


# ------------------------------------------------------------ bass program --
STAGES = ["dis", "xprime", "conv1nr", "conv1", "bn1", "ag2", "conv2nr", "full"]
DBG = dict(no_gather=False, no_wsel=False, no_blocks=False, no_self=False,
           no_moments=False, no_close=False)


def build(st, stage="full", reps=1):
    slev = STAGES.index(stage)
    cfg = st["cfg"]
    N, F, H, O, NCORE = cfg["N"], cfg["F"], cfg["H"], cfg["O"], cfg["NCORE"]
    HALF, EPS, NEG, YC = cfg["HALF"], cfg["EPS"], cfg["NEG"], cfg["YC"]
    NP, T, LT, NB, NBA, NBB, Cu = (st["NP"], st["T"], st["LT"], st["NB"],
                                   st["NBA"], st["NBB"], st["Cu"])
    W2TOT, woff, Wt = st["W2TOT"], st["woff"], st["Wt"]
    d0s, dends, blk_tile, tsb = st["d0"], st["dend"], st["blk_tile"], st["tsb"]
    HB = N - HALF
    rg = [list(range(NCORE))]

    nc = bacc.Bacc("TRN2", target_bir_lowering=False, debug=False,
                   num_devices=NCORE, num_swdge_queues=4)
    GCH = 1024  # max dma_gather indices per instruction (desc ring capacity)
    NQ = 4
    qctr = [0]  # rotate swdge queues so desc-gen overlaps draining

    # --- I/O ---
    x_full = nc.dram_tensor("x_full", [N, F], FP32, kind="ExternalInput")
    x_own = nc.dram_tensor("x_own", [NP, F], FP32, kind="ExternalInput")
    W1 = nc.dram_tensor("w1", [F, H], FP32, kind="ExternalInput")
    g1 = nc.dram_tensor("g1", [H], FP32, kind="ExternalInput")
    be1 = nc.dram_tensor("beta1", [H], FP32, kind="ExternalInput")
    W2 = nc.dram_tensor("w2m", [H, O], FP32, kind="ExternalInput")
    g2 = nc.dram_tensor("g2", [O], FP32, kind="ExternalInput")
    be2 = nc.dram_tensor("beta2", [O], FP32, kind="ExternalInput")
    idxA_d = nc.dram_tensor("idxA", [128, st["LA"] // 16], I16, kind="ExternalInput")
    idxB_d = nc.dram_tensor("idxB", [128, st["LB"] // 16], I16, kind="ExternalInput")
    did_d = nc.dram_tensor("destid", [128, NB], BF16, kind="ExternalInput")
    wv_d = nc.dram_tensor("wval", [128, NB], FP32, kind="ExternalInput")
    cmp_d = nc.dram_tensor("cmp", [NB, Cu], BF16, kind="ExternalInput")
    w2_d = nc.dram_tensor("w2t", [128, W2TOT], FP32, kind="ExternalInput")
    out_d = nc.dram_tensor("out", [NP, O], FP32, kind="ExternalOutput")

    def bcast_inner(ap, k):
        return bass.AP(tensor=ap.tensor, offset=ap.offset, ap=ap.ap + [[0, k]])

    def bcast_part(src_ap, off, n, parts=128):
        return bass.AP(tensor=src_ap.tensor, offset=src_ap.offset + off,
                       ap=[[0, parts], [1, n]])

    with tile.TileContext(nc) as tc:
        sing = tc.alloc_tile_pool(name="sing", bufs=1)
        small = tc.alloc_tile_pool(name="small", bufs=3)
        gbufA_p = tc.alloc_tile_pool(name="gbufA", bufs=2)
        gbufB_p = tc.alloc_tile_pool(name="gbufB", bufs=2)
        wselp = tc.alloc_tile_pool(name="wselp", bufs=2)
        big = tc.alloc_tile_pool(name="big", bufs=1)
        xsc = tc.alloc_tile_pool(name="xsc", bufs=3)
        ptile = tc.alloc_tile_pool(name="ptile", bufs=2, space="PSUM")
        pmisc = tc.alloc_tile_pool(name="pmisc", bufs=2, space="PSUM")
        pfold = tc.alloc_tile_pool(name="pfold", bufs=1, space="PSUM")
        pmom = tc.alloc_tile_pool(name="pmom", bufs=1, space="PSUM")
        pyc = tc.alloc_tile_pool(name="pyc", bufs=2, space="PSUM")
        dram = tc.alloc_tile_pool(name="dram", bufs=1, space="DRAM")

        # --- persistent DRAM scratch ---
        x_prime = dram.tile([N, F], F16)
        din = dram.tile([((NP + 127) // 128) * 128], FP32)
        dago = dram.tile([((NP + 127) // 128) * 128 * NCORE], FP32)
        dis_full = dram.tile([N], FP32)
        mo_in = dram.tile([128, H + 1], FP32)
        mo_out = dram.tile([128, H + 1], FP32)
        mo2_in = dram.tile([128, H + 1], FP32)
        mo2_out = dram.tile([128, H + 1], FP32)
        x2own_d = dram.tile([NP, F], F16)
        x2full = dram.tile([N, F], F16)

        # --- constants ---
        ident = sing.tile([128, 128], FP32)
        make_identity(nc, ident[:])
        ones_col = sing.tile([128, 1], FP32)
        nc.vector.memset(ones_col[:], 1.0)
        ident_h = sing.tile([128, 128], F16)
        nc.vector.tensor_copy(out=ident_h[:], in_=ident[:])
        zero128 = sing.tile([128, 128], FP32)
        nc.vector.memset(zero128[:], 0.0)
        eps_sb = sing.tile([128, 1], FP32)
        nc.vector.memset(eps_sb[:], EPS)

        W1_sb = sing.tile([F, H], FP32)
        nc.sync.dma_start(out=W1_sb[:], in_=W1[:, :])
        W2_sb = sing.tile([H, O], FP32)
        nc.sync.dma_start(out=W2_sb[:], in_=W2[:, :])
        g1_sb = sing.tile([H, 1], FP32)
        nc.sync.dma_start(out=g1_sb[:], in_=g1[:, None])
        be1_sb = sing.tile([H, 1], FP32)
        nc.sync.dma_start(out=be1_sb[:], in_=be1[:, None])
        g2_sb = sing.tile([O, 1], FP32)
        nc.sync.dma_start(out=g2_sb[:], in_=g2[:, None])
        be2_sb = sing.tile([O, 1], FP32)
        nc.sync.dma_start(out=be2_sb[:], in_=be2[:, None])

        def emit_once():
            # =============== stage 1: degree -> dis ===============
            w2_sb = sing.tile([128, W2TOT], FP32)
            nc.sync.dma_start(out=w2_sb[:], in_=w2_d[:, :])
            deg_sb = sing.tile([128, T], FP32)
            for t in range(T):
                nc.vector.tensor_reduce(
                    out=deg_sb[:, t:t + 1],
                    in_=w2_sb[:, int(woff[t]):int(woff[t + 1])],
                    axis=mybir.AxisListType.X, op=mybir.AluOpType.add)
            sq_sb = sing.tile([128, T], FP32)
            nc.scalar.activation(out=sq_sb[:], in_=deg_sb[:],
                                 func=mybir.ActivationFunctionType.Sqrt)
            dis_sb = sing.tile([128, T], FP32)
            nc.vector.reciprocal(out=dis_sb[:], in_=sq_sb[:])
            # write node-ordered dis_own -> din (via PE transpose: few descs)
            pdt = pmisc.tile([128, 128], FP32, tag="ptr")
            nc.tensor.transpose(pdt[:T, :], dis_sb[:, :T], ident[:])
            disrow = small.tile([128, 128], FP32, tag="disrow")
            nc.vector.tensor_copy(out=disrow[:T], in_=pdt[:T, :])
            if T > 1:
                nc.sync.dma_start(
                    out=bass.AP(tensor=din[:].tensor, offset=din[:].offset,
                                ap=[[128, T - 1], [1, 128]]),
                    in_=disrow[:T - 1, :])
            nc.sync.dma_start(
                out=bass.AP(tensor=din[:].tensor,
                            offset=din[:].offset + 128 * (T - 1),
                            ap=[[0, 1], [1, LT]]),
                in_=disrow[T - 1:T, :LT])
            # zero the tail pad of din
            padn = din.shape[0] - NP
            if padn > 0:
                nc.sync.dma_start(
                    out=bass.AP(tensor=din[:].tensor, offset=din[:].offset + NP,
                                ap=[[1, padn]]),
                    in_=zero128[0:1, :padn])

            nc.gpsimd.collective_compute(
                "AllGather", mybir.AluOpType.bypass, replica_groups=rg,
                ins=[din.opt()], outs=[dago.opt()])
            # repack (strip per-rank pad) into dis_full
            PADP = din.shape[0]
            for c in range(NCORE):
                nc.sync.dma_start(
                    out=bass.AP(tensor=dis_full[:].tensor,
                                offset=dis_full[:].offset + c * NP, ap=[[1, NP]]),
                    in_=bass.AP(tensor=dago[:].tensor,
                                offset=dago[:].offset + c * PADP, ap=[[1, NP]]))

            # disB: [128, NP] broadcast of own dis (for feat-major per-node mults)
            disB = big.tile([128, NP], FP32)
            nc.sync.dma_start(out=disB[:], in_=bcast_part(din[:], 0, NP))

            # dis_sbT [128, NR]: column m holds dis[m*128 + p] (for x'-scale)
            NR = (N + 127) // 128
            LROW = N - 128 * (NR - 1)
            dis_sbT = sing.tile([128, NR], FP32)
            for r in range(0, NR, 128):
                rows = min(128, NR - r)
                full_rows = rows if (r + rows < NR or LROW == 128) else rows - 1
                mt = small.tile([128, 128], FP32, tag="dmt")
                if rows < 128 or full_rows < rows:
                    nc.vector.memset(mt[:], 1.0)
                if full_rows > 0:
                    nc.sync.dma_start(
                        out=mt[:full_rows],
                        in_=bass.AP(tensor=dis_full[:].tensor,
                                    offset=dis_full[:].offset + r * 128,
                                    ap=[[128, full_rows], [1, 128]]))
                if full_rows < rows:
                    nc.sync.dma_start(
                        out=mt[full_rows:full_rows + 1, :LROW],
                        in_=bass.AP(tensor=dis_full[:].tensor,
                                    offset=dis_full[:].offset
                                    + (r + full_rows) * 128,
                                    ap=[[0, 1], [1, LROW]]))
                pdr = pmisc.tile([128, 128], FP32, tag="ptr")
                nc.tensor.transpose(pdr[:], mt[:], ident[:])
                nc.vector.tensor_copy(out=dis_sbT[:, r:r + rows],
                                      in_=pdr[:, :rows])

            # =============== stage 2: x' = x * dis ===============
            ng512 = (N // 512) if slev >= 1 else 0
            for gidx in range(ng512):
                base = gidx * 512
                xt = xsc.tile([128, 4, 128], FP32)
                src = bass.AP(tensor=x_full.ap().tensor, offset=base * F,
                              ap=[[F, 128], [128 * F, 4], [1, F]])
                nc.sync.dma_start(out=xt[:], in_=src)
                xt16 = xsc.tile([128, 4, 128], F16, tag="xt16")
                nc.vector.tensor_tensor(
                    out=xt16[:], in0=xt[:],
                    in1=bcast_inner(dis_sbT[:, gidx * 4:gidx * 4 + 4], 128),
                    op=mybir.AluOpType.mult)
                dst = bass.AP(tensor=x_prime[:].tensor,
                              offset=x_prime[:].offset + base * F,
                              ap=[[F, 128], [128 * F, 4], [1, F]])
                nc.sync.dma_start(out=dst, in_=xt16[:])
            rem = (N - (N // 512) * 512) if slev >= 1 else 0
            base = (N // 512) * 512
            while rem > 0:
                nn_ = min(128, rem)
                xt = xsc.tile([128, 128], FP32)
                nc.sync.dma_start(out=xt[:nn_], in_=x_full[base:base + nn_, :])
                col = base // 128
                xt16 = xsc.tile([128, 128], F16, tag="xt16t")
                nc.vector.tensor_scalar_mul(out=xt16[:nn_], in0=xt[:nn_],
                                            scalar1=dis_sbT[:nn_, col:col + 1])
                nc.sync.dma_start(out=x_prime[base:base + nn_, :], in_=xt16[:nn_])
                base += nn_
                rem -= nn_

            # =============== conv layer (shared emitter) ===============
            def conv(layer, src_dram, self_dram, self_scale, zT, do_ar=True):
                """Aggregate z^T[feat, own-node] * dis  into zT (sbuf [F, NP])."""
                Mp = pmom.tile([128, H + 1], FP32)  # moments accumulate (F x F | sum)
                for g in st["groups"]:
                    t0, t1 = g["t0"], g["t1"]
                    bufs = {}
                    for h, tag, pool, idx_d, Lh in ((0, "A", gbufA_p, idxA_d, st["LA"]),
                                                    (1, "B", gbufB_p, idxB_d, st["LB"])):
                        s0, s1 = g[f"s0{tag}"], g[f"s1{tag}"]
                        nb = (s1 - s0) // 128
                        if nb == 0:
                            continue
                        it = small.tile([128, (s1 - s0) // 16], I16, tag=f"idx{tag}")
                        nc.sync.dma_start(out=it[:], in_=idx_d[:, s0 // 16:s1 // 16])
                        gb = pool.tile([128, nb, 128], F16, tag=f"g{tag}")
                        if h == 0:
                            src_ap = src_dram[0:HALF, :]
                        else:
                            src_ap = src_dram[HALF:N, :]
                        if not DBG["no_gather"]:
                            for o in range(0, s1 - s0, GCH):
                                ni = min(GCH, s1 - s0 - o)
                                nc.gpsimd.dma_gather(
                                    out_ap=gb[:, o // 128:(o + ni) // 128, :],
                                    in_ap=src_ap,
                                    idxs_ap=it[:, o // 16:(o + ni) // 16],
                                    num_idxs=ni, num_idxs_reg=ni, elem_size=F,
                                    queue_num=qctr[0] % NQ)
                                qctr[0] += 1
                        else:
                            nc.vector.memset(gb[:], 0.25)
                        # block metadata for this span
                        cb0 = g[f"b0{tag}"] + (0 if h == 0 else NBA)
                        cb1 = g[f"b1{tag}"] + (0 if h == 0 else NBA)
                        dd = small.tile([128, nb], BF16, tag=f"dd{tag}")
                        nc.sync.dma_start(out=dd[:], in_=did_d[:, cb0:cb1])
                        wv = small.tile([128, nb], FP32, tag=f"wv{tag}")
                        nc.sync.dma_start(out=wv[:], in_=wv_d[:, cb0:cb1])
                        cm = small.tile([128, nb, Cu], BF16, tag=f"cm{tag}")
                        nc.sync.dma_start(
                            out=cm[:],
                            in_=bcast_part(cmp_d[:, :], cb0 * Cu, nb * Cu))
                        ws = wselp.tile([128, nb, Cu], F16, tag=f"ws{tag}")
                        if not DBG["no_wsel"]:
                            nc.vector.tensor_tensor(out=ws[:], in0=cm[:],
                                                    in1=bcast_inner(dd[:], Cu),
                                                    op=mybir.AluOpType.is_equal)
                            nc.vector.tensor_tensor(out=ws[:], in0=ws[:],
                                                    in1=bcast_inner(wv[:], Cu),
                                                    op=mybir.AluOpType.mult)
                        else:
                            nc.vector.memset(ws[:], 0.5)
                        bufs[h] = (gb, ws, g[f"b0{tag}"], nb)

                    for t in range(t0, t1):
                        tn = 128 if t < T - 1 else LT
                        blist = []
                        for h in (0, 1):
                            if h not in bufs:
                                continue
                            gb, ws, bbase, nb = bufs[h]
                            for b in range(int(tsb[h, t]) // 128,
                                           int(tsb[h, t + 1]) // 128):
                                blist.append((gb, ws, b - bbase,
                                              int(d0s[h, b]),
                                              min(int(dends[h, b] - d0s[h, b]),
                                                  Cu, 128 - int(d0s[h, b]))))
                        pz = ptile.tile([128, 128], FP32, tag="pz")
                        # self loop term (start=True resets the accumulation)
                        sv = small.tile([128, 128], F16, tag="selfv")
                        if tn < 128:
                            nc.vector.memset(sv[:], 0.0)
                        if self_scale:
                            svf = small.tile([128, 128], FP32, tag="selfvf")
                            nc.sync.dma_start(
                                out=svf[:tn],
                                in_=self_dram[t * 128:t * 128 + tn, :])
                            nc.scalar.activation(
                                out=sv[:tn], in_=svf[:tn],
                                func=mybir.ActivationFunctionType.Identity,
                                scale=dis_sb[:tn, t:t + 1])
                        else:
                            nc.sync.dma_start(
                                out=sv[:tn],
                                in_=self_dram[t * 128:t * 128 + tn, :])
                        if DBG["no_blocks"]:
                            blist = []
                        if not DBG["no_self"]:
                            nc.tensor.matmul(pz[:], lhsT=sv[:], rhs=ident_h[:],
                                             start=True, stop=(len(blist) == 0),
                                             skip_group_check=True)
                        for i, (gb, ws, j, dd0, cb) in enumerate(blist):
                            nc.tensor.matmul(
                                pz[:, dd0:dd0 + cb],
                                lhsT=gb[:, j, :], rhs=ws[:, j, 0:cb],
                                start=(DBG["no_self"] and i == 0),
                                stop=(i == len(blist) - 1),
                                skip_group_check=True)
                        # close tile: zT[:, range] = psum * disB
                        if not DBG["no_close"]:
                            nc.vector.tensor_tensor(
                                out=zT[:, t * 128:t * 128 + tn], in0=pz[:, :tn],
                                in1=disB[:, t * 128:t * 128 + tn],
                                op=mybir.AluOpType.mult)
                        else:
                            nc.vector.memset(zT[:, t * 128:t * 128 + tn], 0.1)
                        if DBG["no_moments"]:
                            continue
                        # moments: transpose then M += z z^T, S += z^T 1
                        ptr = pmisc.tile([128, 128], FP32, tag="ptr")
                        nc.tensor.transpose(ptr[:tn, :],
                                            zT[:, t * 128:t * 128 + tn], ident[:])
                        zd = small.tile([128, H + 1], FP32, tag="zd")
                        if tn < 128:
                            nc.vector.memset(zd[:], 0.0)
                        nc.vector.memset(zd[:, H:H + 1], 1.0)
                        nc.vector.tensor_copy(out=zd[:tn, 0:H], in_=ptr[:tn, :])
                        nc.tensor.matmul(Mp[:, 0:H + 1], lhsT=zd[:, 0:H],
                                         rhs=zd[:, 0:H + 1],
                                         start=(t == 0), stop=(t == T - 1),
                                         skip_group_check=True)
                if DBG["no_moments"]:
                    return None
                mo_sb = small.tile([128, H + 1], FP32, tag="mo")
                nc.vector.tensor_copy(out=mo_sb[:], in_=Mp[:])
                min_d = mo_in if layer == 1 else mo2_in
                mout_d = mo_out if layer == 1 else mo2_out
                nc.sync.dma_start(out=min_d[:, :], in_=mo_sb[:])
                if not do_ar:
                    return None
                nc.gpsimd.collective_compute(
                    "AllReduce", mybir.AluOpType.add, replica_groups=rg,
                    ins=[min_d.opt()], outs=[mout_d.opt()])
                mg = small.tile([128, H + 1], FP32, tag="mg")
                nc.sync.dma_start(out=mg[:], in_=mout_d[:, :])
                return mg

            def bn_fold(mg, Wsb, HH, g_sb, be_sb):
                """-> (s, tb) per-feature scale/shift [HH, 1] from moments."""
                pf = pfold.tile([128, 128], FP32, tag="pf")
                # mu_lin = W^T S / N
                nc.tensor.matmul(pf[:HH, 0:1], lhsT=Wsb[:], rhs=mg[:, H:H + 1],
                                 start=True, stop=True, skip_group_check=True)
                mul_sb = small.tile([128, 1], FP32, tag="mul")
                nc.vector.tensor_scalar_mul(out=mul_sb[:HH], in0=pf[:HH, 0:1],
                                            scalar1=1.0 / N)
                # G = M W ; diag = sum_f W*G
                pg = pfold.tile([128, 128], FP32, tag="pf")
                nc.tensor.matmul(pg[:, 0:HH], lhsT=mg[:, 0:H], rhs=Wsb[:],
                                 start=True, stop=True, skip_group_check=True)
                wg = small.tile([128, HH], FP32, tag="wg")
                nc.vector.tensor_tensor(out=wg[:], in0=pg[:, 0:HH], in1=Wsb[:],
                                        op=mybir.AluOpType.mult)
                pd = pfold.tile([128, 128], FP32, tag="pf")
                nc.tensor.matmul(pd[:HH, 0:1], lhsT=wg[:], rhs=ones_col[:],
                                 start=True, stop=True, skip_group_check=True)
                var_sb = small.tile([128, 1], FP32, tag="var")
                nc.vector.tensor_scalar_mul(out=var_sb[:HH], in0=pd[:HH, 0:1],
                                            scalar1=1.0 / N)
                mu2 = small.tile([128, 1], FP32, tag="mu2")
                nc.vector.tensor_mul(mu2[:HH], mul_sb[:HH], mul_sb[:HH])
                nc.vector.tensor_sub(var_sb[:HH], var_sb[:HH], mu2[:HH])
                sqv = small.tile([128, 1], FP32, tag="sqv")
                nc.scalar.activation(out=sqv[:HH], in_=var_sb[:HH],
                                     func=mybir.ActivationFunctionType.Sqrt,
                                     bias=eps_sb[:HH])
                s_sb = small.tile([128, 1], FP32, tag="s")
                nc.vector.reciprocal(out=s_sb[:HH], in_=sqv[:HH])
                nc.vector.tensor_mul(s_sb[:HH], s_sb[:HH], g_sb[:HH])
                tb_sb = small.tile([128, 1], FP32, tag="tb")
                nc.vector.tensor_mul(tb_sb[:HH], mul_sb[:HH], s_sb[:HH])
                nc.vector.tensor_sub(tb_sb[:HH], be_sb[:HH], tb_sb[:HH])
                return s_sb, tb_sb

            # ---- layer 1 ----
            if slev >= 2:
                zT1 = big.tile([128, NP], FP32, tag="zbig")
                mg1 = conv(1, x_prime, x_own, True, zT1, do_ar=(slev >= 3))
            if slev >= 4:
                s1, tb1 = bn_fold(mg1, W1_sb, H, g1_sb, be1_sb)
            for c0 in range(0, NP, YC) if slev >= 4 else []:
                c1 = min(c0 + YC, NP)
                py = pyc.tile([128, YC], FP32, tag="py")
                nc.tensor.matmul(py[:, 0:c1 - c0], lhsT=W1_sb[:], rhs=zT1[:, c0:c1],
                                 start=True, stop=True, skip_group_check=True)
                u = small.tile([128, YC], FP32, tag="u")
                nc.scalar.activation(out=u[:, 0:c1 - c0], in_=py[:, 0:c1 - c0],
                                     func=mybir.ActivationFunctionType.Identity,
                                     scale=s1[:H], bias=tb1[:H])
                v = small.tile([128, YC], FP32, tag="v")
                nc.vector.tensor_scalar_mul(out=v[:, 0:c1 - c0], in0=u[:, 0:c1 - c0],
                                            scalar1=NEG)
                nc.vector.tensor_tensor(out=u[:, 0:c1 - c0], in0=u[:, 0:c1 - c0],
                                        in1=v[:, 0:c1 - c0], op=mybir.AluOpType.max)
                nc.vector.tensor_tensor(out=u[:, 0:c1 - c0], in0=u[:, 0:c1 - c0],
                                        in1=disB[:, c0:c1], op=mybir.AluOpType.mult)
                # x2' rows -> dram (AG input), per 128-node tile of this chunk
                for tb_ in range(c0 // 128, (c1 + 127) // 128):
                    n0 = tb_ * 128
                    tn = min(128, NP - n0)
                    ptr = pmisc.tile([128, 128], FP32, tag="ptr")
                    nc.tensor.transpose(ptr[:tn, :], u[:, n0 - c0:n0 - c0 + tn],
                                        ident[:])
                    xo = small.tile([128, 128], F16, tag="xo")
                    nc.vector.tensor_copy(out=xo[:tn], in_=ptr[:tn, :])
                    nc.sync.dma_start(out=x2own_d[n0:n0 + tn, :], in_=xo[:tn])
            if slev >= 5:
                nc.gpsimd.collective_compute(
                    "AllGather", mybir.AluOpType.bypass, replica_groups=rg,
                    ins=[x2own_d.opt()], outs=[x2full.opt()])

            # ---- layer 2 ----
            if slev >= 6:
                zT2 = big.tile([128, NP], FP32, tag="zbig")
                mg2 = conv(2, x2full, x2own_d, False, zT2, do_ar=(slev >= 7))
            if slev >= 7:
                s2, tb2 = bn_fold(mg2, W2_sb, O, g2_sb, be2_sb)
            for c0 in range(0, NP, YC) if slev >= 7 else []:
                c1 = min(c0 + YC, NP)
                py = pyc.tile([128, YC], FP32, tag="py")
                nc.tensor.matmul(py[:O, 0:c1 - c0], lhsT=W2_sb[:], rhs=zT2[:, c0:c1],
                                 start=True, stop=True, skip_group_check=True)
                u = small.tile([128, YC], FP32, tag="u")
                nc.scalar.activation(out=u[:O, 0:c1 - c0], in_=py[:O, 0:c1 - c0],
                                     func=mybir.ActivationFunctionType.Identity,
                                     scale=s2[:O], bias=tb2[:O])
                # transpose per 128-node tile and write out
                for tb_ in range(c0 // 128, (c1 + 127) // 128):
                    n0 = tb_ * 128
                    tn = min(128, NP - n0)
                    po = pmisc.tile([128, 128], FP32, tag="ptr")
                    nc.tensor.transpose(po[:tn, :O], u[:O, n0 - c0:n0 - c0 + tn],
                                        ident[:O, :O])
                    oo = small.tile([128, O], FP32, tag="oo")
                    nc.vector.tensor_copy(out=oo[:tn], in_=po[:tn, :O])
                    nc.sync.dma_start(out=out_d[n0:n0 + tn, :], in_=oo[:tn])


        for _rep in range(reps):
            emit_once()

        for p in (dram, pyc, pmom, pfold, pmisc, ptile, xsc, big, wselp,
                  gbufB_p, gbufA_p, small, sing):
            p.release()

    nc.compile()
    return nc


# ------------------------------------------------------------------ runner --
def make_in_maps(st, inputs):
    cfg = st["cfg"]
    N, NCORE = cfg["N"], cfg["NCORE"]
    NP = st["NP"]
    x = np.asarray(inputs["drug_smiles_fea"], np.float32)
    maps = []
    for c in range(NCORE):
        maps.append(dict(
            x_full=x,
            x_own=np.ascontiguousarray(x[c * NP:(c + 1) * NP]),
            w1=np.asarray(inputs["W1"], np.float32),
            g1=np.asarray(inputs["g1"], np.float32),
            beta1=np.asarray(inputs["beta1"], np.float32),
            w2m=np.asarray(inputs["W2"], np.float32),
            g2=np.asarray(inputs["g2"], np.float32),
            beta2=np.asarray(inputs["beta2"], np.float32),
            idxA=st["idxA"][c], idxB=st["idxB"][c],
            destid=np.ascontiguousarray(st["destid"][c]),
            wval=np.ascontiguousarray(st["wval"][c]),
            cmp=st["cmp"],
            w2t=np.ascontiguousarray(st["w2"][c]),
        ))
    return maps


_LAST = {}


def kernel(**inputs):
    cfg = CFG
    adj = np.asarray(inputs["ATC_adj"])
    w = np.asarray(inputs["ATC_weight"], np.float32)
    st = preprocess(adj, w, cfg)
    nc = build(st)
    maps = make_in_maps(st, inputs)
    res = bass_utils.run_bass_kernel_spmd(
        nc, maps, core_ids=list(range(cfg["NCORE"])))
    out = np.concatenate([res.results[c]["out"] for c in range(cfg["NCORE"])], 0)
    _LAST.update(st=st, nc=nc, maps=maps)
    return out

